# revision 14
# baseline (speedup 1.0000x reference)
"""MinibatchDiscrimination TRN2 kernel.

x: [512, 1024] f32, T: [1024, 1024] f32.
M = (x @ T).reshape(512, 64, 16); l1[i,j,k] = sum_d |M[i,k,d]-M[j,k,d]|
out[i,k] = sum_j exp(-l1[i,j,k]) - 1.

Sharding: batch rows i split across 8 cores (64 each). Every core computes the
full M^T = T^T x^T on-chip (cheap), so no collectives are needed. Each core's
copy of x^T has its j-columns rolled so its own 64 rows sit at columns 0..63;
the j-reduction is permutation invariant.

Per-core pipeline:
  phase 1: M^T [kd=1024, j=512] = T.T @ x.T on PE (f32r full-rate), 8 SBUF
           tiles [128, 512].
  phase 2 (per local i): abs tiles |M^T - M^T[:, i]| via DVE tensor_scalar
           (subtract, abs_max 0) and ACT activation(Abs, scale=-1, bias=col);
           d-sum via PE matmul with a one-hot S [128, 64] per kd-tile into
           PSUM l1 [64 k, 512 j]; ACT Exp(scale=-1) with accum_out fusing the
           j-sum into an output column.
  host: transpose [k, i] -> [i, k], subtract 1, concatenate cores.
"""

import numpy as np

import concourse.bass as bass
import concourse.tile as tile
from concourse import mybir
from concourse import bass_utils
from concourse.vector_clock import ScopedClock

B = 512
F = 1024
KD = 1024  # = NUM_KERNELS(64) * KERNEL_DIM(16)
NK = 64
DD = 16
N_CORES = 8
NI = B // N_CORES  # local rows per core
NT = KD // 128  # kd tiles
NF = F // 128  # f chunks
# abs-tile engine split: t < N_DVE on VectorE, rest on ScalarE
N_DVE = 6

_FP32 = mybir.dt.float32
_F32R = mybir.dt.float32r


def _split_all_waits(nc):
    """walrus in this env encodes at most 1 sync wait per instruction: hoist
    extra waits onto same-engine NOPs inserted just before the instruction.
    Safe because waits are AND-ed stall conditions on the engine's sequencer
    and semaphores are monotonic."""
    count = 0
    for fn in nc.m.functions:
        for bb in fn.blocks:
            insts = list(bb.instructions)
            new = []
            changed = False
            for inst in insts:
                si = getattr(inst, "sync_info", None)
                waits = list(si.on_wait) if (si is not None and si.on_wait) else []
                if len(waits) > 1:
                    for w in waits[:-1]:
                        nop = mybir.InstNoOp(name=f"NOPW-{count}", ins=[], outs=[])
                        count += 1
                        nop.engine = inst.engine
                        nop.sync_info = mybir.SyncInfo(on_wait=[w], on_update=[])
                        nc.register_instruction(nop, overwrite=True)
                        new.append(nop)
                    si.on_wait = [waits[-1]]
                    changed = True
                new.append(inst)
            if changed:
                bb.instructions[:] = new


def _patch_drain_wait_limit():
    if getattr(tile.TileContext, "_wait_split_patched", False):
        return
    orig = tile.TileContext.schedule_and_allocate

    def schedule_and_allocate(self, *a, **k):
        r = orig(self, *a, **k)
        _split_all_waits(self.nc)
        return r

    tile.TileContext.schedule_and_allocate = schedule_and_allocate
    tile.TileContext._wait_split_patched = True


def build_s_matrices():
    """One-hot d-sum matrices per kd-tile t (tile t holds k in [8t, 8t+8)):
    base S_t[p, m] = 1 iff m == t*8 + p//16.

    |z| = 2*relu(z) - z (DVE tiles, s=+1) or |z| = 2*relu(-z) + z (ACT tiles,
    s=-1), so l1 = 2*sum relu - s*(G - g_i) with G[k,j] = sum_d M[j,kd].
    S2 = 2*S_t for the relu matmuls; Sg = -s_t*S_t builds Gsneg = -s*G.
    I64 is the G-injection lhsT."""
    S2 = np.zeros((128, NT * NK), dtype=np.float32)
    Sg = np.zeros((128, NT * NK), dtype=np.float32)
    for t in range(NT):
        s_t = 1.0 if t < N_DVE else -1.0
        for p in range(128):
            m = t * NK + t * 8 + p // 16
            S2[p, m] = 2.0
            Sg[p, m] = -s_t
    I64 = np.eye(NK, dtype=np.float32)
    return S2, Sg, I64


def build_program(repeat: int = 1):
    _patch_drain_wait_limit()
    nc = bass.Bass(
        "TRN2", target_bir_lowering=False, debug=False, num_devices=N_CORES
    )
    xT_d = nc.dram_tensor("xT", [F, B], _FP32, kind="ExternalInput").ap()
    T_d = nc.dram_tensor("T", [F, KD], _FP32, kind="ExternalInput").ap()
    S2_d = nc.dram_tensor("S2", [128, NT * NK], _FP32, kind="ExternalInput").ap()
    Sg_d = nc.dram_tensor("Sg", [128, NT * NK], _FP32, kind="ExternalInput").ap()
    I64_d = nc.dram_tensor("I64", [NK, NK], _FP32, kind="ExternalInput").ap()
    o_d = nc.dram_tensor("o", [NK, NI], _FP32, kind="ExternalOutput").ap()

    AF = mybir.ActivationFunctionType
    import contextlib

    with tile.TileContext(nc) as tc:
        with (
            tc.tile_pool(name="tsb", bufs=NF) as t_pool,
            tc.tile_pool(name="xsb", bufs=NF) as x_pool,
            tc.tile_pool(name="mt", bufs=NT) as mt_pool,
            tc.tile_pool(name="ssb", bufs=1) as s_pool,
            tc.tile_pool(name="absp", bufs=16) as abs_pool,
            tc.tile_pool(name="dum", bufs=4) as dum_pool,
            tc.tile_pool(name="op", bufs=1) as o_pool,
            tc.tile_pool(name="pmm", bufs=2, space="PSUM") as psum_mm,
            tc.tile_pool(name="pl1", bufs=6, space="PSUM") as psum_l1,
            tc.For_i(0, repeat, 1) if repeat > 1 else contextlib.nullcontext(),
        ):
            # f32r matmul inputs must be produced pre-rounded to f32r, so DMA
            # fp32 then round via a DVE copy into f32r-typed tiles.
            T_sb = []
            for f in range(NF):
                tt = t_pool.tile([128, KD], _FP32, tag="tsb")
                nc.sync.dma_start(out=tt, in_=T_d[f * 128 : (f + 1) * 128, :])
                tr = t_pool.tile([128, KD], _F32R, tag="tr")
                nc.vector.tensor_copy(tr, tt)
                T_sb.append(tr)
            x_sb = []
            for f in range(NF):
                xt = x_pool.tile([128, B], _FP32, tag="xsb")
                nc.sync.dma_start(out=xt, in_=xT_d[f * 128 : (f + 1) * 128, :])
                xr = x_pool.tile([128, B], _F32R, tag="xr")
                nc.vector.tensor_copy(xr, xt)
                x_sb.append(xr)
            S2_f = s_pool.tile([128, NT * NK], _FP32, tag="s2f")
            nc.sync.dma_start(out=S2_f, in_=S2_d)
            S2_sb = s_pool.tile([128, NT * NK], _F32R, tag="s2r")
            nc.vector.tensor_copy(S2_sb, S2_f)
            Sg_f = s_pool.tile([128, NT * NK], _FP32, tag="sgf")
            nc.sync.dma_start(out=Sg_f, in_=Sg_d)
            Sg_sb = s_pool.tile([128, NT * NK], _F32R, tag="sgr")
            nc.vector.tensor_copy(Sg_sb, Sg_f)
            I64_f = s_pool.tile([NK, NK], _FP32, tag="i64f")
            nc.sync.dma_start(out=I64_f, in_=I64_d)
            I64_sb = s_pool.tile([NK, NK], _F32R, tag="i64r")
            nc.vector.tensor_copy(I64_sb, I64_f)

            # phase 1: M^T tiles [128 kd, 512 j]
            mt = []
            mt_r = []
            for t in range(NT):
                pm = psum_mm.tile([128, B], _FP32, tag="pmm")
                for f in range(NF):
                    nc.tensor.matmul(
                        pm,
                        lhsT=T_sb[f][:, t * 128 : (t + 1) * 128],
                        rhs=x_sb[f],
                        start=(f == 0),
                        stop=(f == NF - 1),
                    )
                m = mt_pool.tile([128, B], _FP32, tag="mt")
                nc.scalar.copy(m, pm)
                mr = mt_pool.tile([128, B], _F32R, tag="mtr")
                nc.vector.tensor_copy(mr, pm)
                mt.append(m)
                mt_r.append(mr)

            # phase 1.5: Gsneg[k, j] = -s_k * sum_d M[j, kd]
            pg = psum_mm.tile([NK, B], _FP32, tag="pmm")
            for t in range(NT):
                nc.tensor.matmul(
                    pg,
                    lhsT=Sg_sb[:, t * NK : (t + 1) * NK],
                    rhs=mt_r[t],
                    start=(t == 0),
                    stop=(t == NT - 1),
                )
            Gsneg = s_pool.tile([NK, B], _F32R, tag="gsneg")
            nc.vector.tensor_copy(Gsneg, pg)

            O_t = o_pool.tile([NK, NI], _FP32)

            # phase 2: l1_acc = sum_t 2*S_t.T @ relu_t  -  s*G;
            # exp(-l1) = Exp(-l1_acc + Gsneg[:, i]) summed over j via accum_out
            for i in range(NI):
                relus = []
                for t in range(NT):
                    ab = abs_pool.tile([128, B], _F32R, tag="absp")
                    col = mt[t][:, i : i + 1]
                    if t < N_DVE:
                        nc.vector.tensor_scalar(
                            ab,
                            mt[t],
                            col,
                            0.0,
                            op0=mybir.AluOpType.subtract,
                            op1=mybir.AluOpType.max,
                        )
                    else:
                        nc.scalar.activation(ab, mt[t], AF.Relu, bias=col, scale=-1.0)
                    relus.append(ab)
                l1 = psum_l1.tile([NK, B], _FP32, tag="pl1")
                for t in range(NT):
                    nc.tensor.matmul(
                        l1,
                        lhsT=S2_sb[:, t * NK : (t + 1) * NK],
                        rhs=relus[t],
                        start=(t == 0),
                        stop=False,
                    )
                nc.tensor.matmul(l1, lhsT=I64_sb, rhs=Gsneg, start=False, stop=True)
                dum = dum_pool.tile([NK, B], mybir.dt.bfloat16, tag="dum")
                nc.scalar.activation(
                    dum, l1, AF.Exp,
                    bias=Gsneg[:, i : i + 1].bitcast(_FP32),
                    scale=-1.0,
                    accum_out=O_t[:, i : i + 1],
                )

            nc.sync.dma_start(out=o_d, in_=O_t)
    return nc


_CACHED = {}


def _get_program(repeat: int = 1):
    key = f"nc{repeat}"
    if key not in _CACHED:
        _CACHED[key] = build_program(repeat)
        _CACHED["S"] = build_s_matrices()
    return _CACHED[key], _CACHED["S"]


def run(x: np.ndarray, T: np.ndarray, trace: bool = False, repeat: int = 1):
    nc, (S2, Sg, I64) = _get_program(repeat)
    xT = np.ascontiguousarray(x.T.astype(np.float32, copy=False))
    T_arr = np.ascontiguousarray(T.astype(np.float32, copy=False))
    in_maps = []
    for c in range(N_CORES):
        xTc = np.ascontiguousarray(np.roll(xT, -NI * c, axis=1))
        in_maps.append({"xT": xTc, "T": T_arr, "S2": S2, "Sg": Sg, "I64": I64})
    res = bass_utils.run_bass_kernel_spmd(
        nc, in_maps, core_ids=list(range(N_CORES)), trace=trace
    )
    out = np.concatenate(
        [res.results[c]["o"].T - 1.0 for c in range(N_CORES)], axis=0
    ).astype(np.float32)
    return out, res


def kernel(x: np.ndarray, T: np.ndarray) -> np.ndarray:
    out, _ = run(x, T)
    return out


# revision 15
# speedup vs baseline: 1.6345x; 1.6345x over previous
"""MinibatchDiscrimination TRN2 kernel.

x: [512, 1024] f32, T: [1024, 1024] f32.
M = (x @ T).reshape(512, 64, 16); l1[i,j,k] = sum_d |M[i,k,d]-M[j,k,d]|
out[i,k] = sum_j exp(-l1[i,j,k]) - 1.

Sharding: batch rows split across 8 cores (64 each), no collectives. Each
core's copy of x^T has its j-columns rolled so its own 64 rows sit at local
columns 0..63.

Symmetric pair coverage: core-local row i computes the j-window
[i+1, i+257) (pair distance d in [1, 256]). Over all cores/rows every
unordered pair {a, a+d} with d in [1, 255] is computed exactly once (its
exp(-l1) is accumulated both to row a via the activation accum_out and to
row a+d via a transposed column accumulator), while d = 256 pairs are
computed twice (once from each end) and accumulated row-side only. The
diagonal is never computed, so no -1 correction is needed.

|z| via relu: DVE tiles use relu(z) = (z sub m) max 0 (s=+1), ACT tiles use
Relu(-z) (s=-1); l1 = 2*sum relu - s*(G[k,j] - G[k,i]) with
G[k,j] = sum_d M[j,kd]. The -s*G[k,j] term is injected into the PSUM
accumulation via one extra matmul (lhsT=I64, rhs=Gsneg slice) and the
+s*G[k,i] term rides the exp's per-partition bias (bias = Gsneg[:, i]).

f32r everywhere on the PE (full-rate; fp32 matmul is 1/4 rate). The BIR
verifier requires f32r matmul operands to be produced pre-rounded, so DMA'd
fp32 data is rounded on-chip via DVE copies into f32r tiles.
"""

import contextlib

import numpy as np

import concourse.bass as bass
import concourse.tile as tile
from concourse import mybir
from concourse import bass_utils

B = 512
F = 1024
KD = 1024  # = NUM_KERNELS(64) * KERNEL_DIM(16)
NK = 64
N_CORES = 8
NI = B // N_CORES  # local rows per core
NT = KD // 128  # kd tiles
NF = F // 128  # f chunks
W = 256  # j-window width per row
JL = NI + W  # used local-j extent
LAG = 6  # transpose-add lag (iterations) to keep DVE from stalling on exp
# relu-tile engine split: t < N_DVE on VectorE (s=+1), rest on ScalarE (s=-1)
N_DVE = 5

_FP32 = mybir.dt.float32
_F32R = mybir.dt.float32r


def _split_all_waits(nc):
    """walrus in this env encodes at most 1 sync wait per instruction: hoist
    extra waits onto same-engine NOPs inserted just before the instruction.
    Safe because waits are AND-ed stall conditions on the engine's sequencer
    and semaphores are monotonic."""
    count = 0
    for fn in nc.m.functions:
        for bb in fn.blocks:
            insts = list(bb.instructions)
            new = []
            changed = False
            for inst in insts:
                si = getattr(inst, "sync_info", None)
                waits = list(si.on_wait) if (si is not None and si.on_wait) else []
                if len(waits) > 1:
                    for w in waits[:-1]:
                        nop = mybir.InstNoOp(name=f"NOPW-{count}", ins=[], outs=[])
                        count += 1
                        nop.engine = inst.engine
                        nop.sync_info = mybir.SyncInfo(on_wait=[w], on_update=[])
                        nc.register_instruction(nop, overwrite=True)
                        new.append(nop)
                    si.on_wait = [waits[-1]]
                    changed = True
                new.append(inst)
            if changed:
                bb.instructions[:] = new


def _patch_drain_wait_limit():
    if getattr(tile.TileContext, "_wait_split_patched", False):
        return
    orig = tile.TileContext.schedule_and_allocate

    def schedule_and_allocate(self, *a, **k):
        r = orig(self, *a, **k)
        _split_all_waits(self.nc)
        return r

    tile.TileContext.schedule_and_allocate = schedule_and_allocate
    tile.TileContext._wait_split_patched = True


def build_s_matrices():
    """One-hot d-sum matrices per kd-tile t (tile t holds k in [8t, 8t+8)):
    base S_t[p, m] = 1 iff m == t*8 + p//16. S2 = 2*S_t (relu matmuls),
    Sg = -s_t*S_t (builds Gsneg = -s*G), I64 = identity (G injection)."""
    S2 = np.zeros((128, NT * NK), dtype=np.float32)
    Sg = np.zeros((128, NT * NK), dtype=np.float32)
    for t in range(NT):
        s_t = 1.0 if t < N_DVE else -1.0
        for p in range(128):
            m = t * NK + t * 8 + p // 16
            S2[p, m] = 2.0
            Sg[p, m] = -s_t
    I64 = np.eye(NK, dtype=np.float32)
    return S2, Sg, I64


def build_program(repeat: int = 1):
    _patch_drain_wait_limit()
    nc = bass.Bass(
        "TRN2", target_bir_lowering=False, debug=False, num_devices=N_CORES
    )
    xT_d = nc.dram_tensor("xT", [F, JL], _FP32, kind="ExternalInput").ap()
    T_d = nc.dram_tensor("T", [F, KD], _FP32, kind="ExternalInput").ap()
    S2_d = nc.dram_tensor("S2", [128, NT * NK], _FP32, kind="ExternalInput").ap()
    Sg_d = nc.dram_tensor("Sg", [128, NT * NK], _FP32, kind="ExternalInput").ap()
    I64_d = nc.dram_tensor("I64", [NK, NK], _FP32, kind="ExternalInput").ap()
    orow_d = nc.dram_tensor("orow", [NK, NI], _FP32, kind="ExternalOutput").ap()
    ocol_d = nc.dram_tensor("ocol", [NK, JL], _FP32, kind="ExternalOutput").ap()

    AF = mybir.ActivationFunctionType
    AO = mybir.AluOpType

    with tile.TileContext(nc) as tc:
        with (
            tc.tile_pool(name="stage", bufs=3) as stage,
            tc.tile_pool(name="tr", bufs=NF) as tr_pool,
            tc.tile_pool(name="xr", bufs=NF) as xr_pool,
            tc.tile_pool(name="mt", bufs=NT) as mt_pool,
            tc.tile_pool(name="ssb", bufs=1) as s_pool,
            tc.tile_pool(name="relu", bufs=16) as relu_pool,
            tc.tile_pool(name="ep", bufs=LAG + 3) as e_pool,
            tc.tile_pool(name="op", bufs=1) as o_pool,
            tc.tile_pool(name="pmm", bufs=2, space="PSUM") as psum_mm,
            tc.tile_pool(name="pl1", bufs=6, space="PSUM") as psum_l1,
            tc.For_i(0, repeat, 1) if repeat > 1 else contextlib.nullcontext(),
        ):
            # ---- loads + f32r rounding ----
            T_r = []
            for f in range(NF):
                st = stage.tile([128, KD], _FP32, tag="stage")
                nc.sync.dma_start(out=st, in_=T_d[f * 128 : (f + 1) * 128, :])
                tr = tr_pool.tile([128, KD], _F32R, tag="tr")
                nc.vector.tensor_copy(tr, st)
                T_r.append(tr)
            x_r = []
            for f in range(NF):
                st = stage.tile([128, JL], _FP32, tag="xstage")
                nc.sync.dma_start(out=st, in_=xT_d[f * 128 : (f + 1) * 128, :])
                xr = xr_pool.tile([128, JL], _F32R, tag="xr")
                nc.vector.tensor_copy(xr, st)
                x_r.append(xr)
            st = stage.tile([128, NT * NK], _FP32, tag="s2stage")
            nc.sync.dma_start(out=st, in_=S2_d)
            S2_r = s_pool.tile([128, NT * NK], _F32R, tag="s2r")
            nc.vector.tensor_copy(S2_r, st)
            st = stage.tile([128, NT * NK], _FP32, tag="sgstage")
            nc.sync.dma_start(out=st, in_=Sg_d)
            Sg_r = s_pool.tile([128, NT * NK], _F32R, tag="sgr")
            nc.vector.tensor_copy(Sg_r, st)
            st = stage.tile([NK, NK], _FP32, tag="i64stage")
            nc.sync.dma_start(out=st, in_=I64_d)
            I64_r = s_pool.tile([NK, NK], _F32R, tag="i64r")
            nc.vector.tensor_copy(I64_r, st)

            # ---- phase 1: M^T tiles [128 kd, JL j] (f32r) ----
            mt = []
            for t in range(NT):
                pm = psum_mm.tile([128, JL], _FP32, tag="pmm")
                for f in range(NF):
                    nc.tensor.matmul(
                        pm,
                        lhsT=T_r[f][:, t * 128 : (t + 1) * 128],
                        rhs=x_r[f],
                        start=(f == 0),
                        stop=(f == NF - 1),
                    )
                m = mt_pool.tile([128, JL], _F32R, tag="mt")
                if t % 2 == 0:
                    nc.vector.tensor_copy(m, pm)
                else:
                    nc.scalar.copy(m, pm)
                mt.append(m)

            # ---- phase 1.5: Gsneg[k, j] = -s_k * sum_d M[j, kd] ----
            pg = psum_mm.tile([NK, JL], _FP32, tag="pmm")
            for t in range(NT):
                nc.tensor.matmul(
                    pg,
                    lhsT=Sg_r[:, t * NK : (t + 1) * NK],
                    rhs=mt[t],
                    start=(t == 0),
                    stop=(t == NT - 1),
                )
            Gsneg = s_pool.tile([NK, JL], _F32R, tag="gsneg")
            nc.vector.tensor_copy(Gsneg, pg)

            O_row = o_pool.tile([NK, NI], _FP32, tag="orow")
            O_col = o_pool.tile([NK, JL], _FP32, tag="ocol")
            nc.vector.memset(O_col, 0.0)

            # ---- phase 2 ----
            e_tiles = [None] * NI

            def emit_col_add(j):
                nc.vector.tensor_add(
                    O_col[:, j + 1 : j + W],
                    O_col[:, j + 1 : j + W],
                    e_tiles[j][:, 0 : W - 1],
                )

            for i in range(NI):
                w0, w1 = i + 1, i + 1 + W
                relus = []
                for t in range(NT):
                    ab = relu_pool.tile([128, W], _F32R, tag="relu")
                    col = mt[t][:, i : i + 1].bitcast(_FP32)
                    if t < N_DVE:
                        nc.vector.tensor_scalar(
                            ab, mt[t][:, w0:w1], col, 0.0,
                            op0=AO.subtract, op1=AO.max,
                        )
                    else:
                        nc.scalar.activation(
                            ab, mt[t][:, w0:w1], AF.Relu, bias=col, scale=-1.0
                        )
                    relus.append(ab)
                l1 = psum_l1.tile([NK, W], _FP32, tag="pl1")
                for t in range(NT):
                    nc.tensor.matmul(
                        l1,
                        lhsT=S2_r[:, t * NK : (t + 1) * NK],
                        rhs=relus[t],
                        start=(t == 0),
                        stop=False,
                    )
                nc.tensor.matmul(
                    l1, lhsT=I64_r, rhs=Gsneg[:, w0:w1], start=False, stop=True
                )
                E = e_pool.tile([NK, W], _FP32, tag="ep")
                nc.scalar.activation(
                    E, l1, AF.Exp,
                    bias=Gsneg[:, i : i + 1].bitcast(_FP32),
                    scale=-1.0,
                    accum_out=O_row[:, i : i + 1],
                )
                e_tiles[i] = E
                if i >= LAG:
                    emit_col_add(i - LAG)
            for j in range(NI - LAG, NI):
                emit_col_add(j)

            nc.sync.dma_start(out=orow_d, in_=O_row)
            nc.sync.dma_start(out=ocol_d, in_=O_col)
    return nc


_CACHED = {}


def _get_program(repeat: int = 1):
    key = f"nc{repeat}"
    if key not in _CACHED:
        _CACHED[key] = build_program(repeat)
        _CACHED["S"] = build_s_matrices()
    return _CACHED[key], _CACHED["S"]


def make_in_maps(x: np.ndarray, T: np.ndarray, S2, Sg, I64):
    xT = np.ascontiguousarray(x.T.astype(np.float32, copy=False))
    T_arr = np.ascontiguousarray(T.astype(np.float32, copy=False))
    in_maps = []
    for c in range(N_CORES):
        xTc = np.ascontiguousarray(np.roll(xT, -NI * c, axis=1)[:, :JL])
        in_maps.append({"xT": xTc, "T": T_arr, "S2": S2, "Sg": Sg, "I64": I64})
    return in_maps


def assemble(results) -> np.ndarray:
    out = np.zeros((B, NK), dtype=np.float64)
    for c in range(N_CORES):
        R = results[c]["orow"]  # [k, i_local]
        C = results[c]["ocol"]  # [k, j_local]
        out[NI * c : NI * (c + 1), :] += R.T
        Cfull = np.zeros((B, NK), dtype=np.float64)
        Cfull[:JL] = C.T
        out += np.roll(Cfull, NI * c, axis=0)
    return out.astype(np.float32)


def run(x: np.ndarray, T: np.ndarray, trace: bool = False, repeat: int = 1):
    nc, (S2, Sg, I64) = _get_program(repeat)
    in_maps = make_in_maps(x, T, S2, Sg, I64)
    res = bass_utils.run_bass_kernel_spmd(
        nc, in_maps, core_ids=list(range(N_CORES)), trace=trace
    )
    return assemble(res.results), res


def kernel(x: np.ndarray, T: np.ndarray) -> np.ndarray:
    out, _ = run(x, T)
    return out
